# revision 8
# baseline (speedup 1.0000x reference)
"""GraphSage 3-layer GNN on 8 TRN2 NeuronCores (Bass/Tile) — v2.

Design (per core, nodes sharded 8-way with balanced assignment):
- x for the neighbor path lives in bf16 "pair rows" ([n/2, 128] = 2 nodes per
  256B row) so dma_gather's 256B-elem restriction is met at half the bytes.
- Aggregation: gather pair rows of edge sources into a slab; per tile of 128
  edges build a narrow bf16 selector [128, w] = is_equal(iota, dst_slot -
  band_off) * inv_deg[dst] on DVE (slot-sorted tiles span a narrow dst band),
  then matmul slab_half.T @ sel accumulating into a PSUM region [64, 256]
  (one block-pair of 256 dst nodes). inv_deg folded into the selector.
- x_full is split into NWIN windows (int16 gather idx limit + AllGather
  staggering): window w's AllGather fires as soon as its blocks are done, and
  phase w of the NEXT layer only needs window w — collectives pipeline under
  compute. Layer 0 reads a host-shipped bf16 pair tensor.
- Root path (x @ w_r), residual and fc run in bf16 with fp32 PSUM; LayerNorm
  statistics and normalization run in fp32.
"""

import hashlib
import numpy as np
import ml_dtypes
from contextlib import ExitStack

import jax

# persistent XLA/NEFF compile cache: makes fresh-process first calls cheap
try:
    jax.config.update("jax_compilation_cache_dir", "/tmp/jax_neff_cache")
    jax.config.update("jax_persistent_cache_min_compile_time_secs", 0.0)
except Exception:
    pass

import concourse.bass as bass
import concourse.bacc as bacc
import concourse.tile as tile
from concourse import mybir
from concourse.bass_utils import run_bass_kernel_spmd

F32 = mybir.dt.float32
BF16 = mybir.dt.bfloat16
I16 = mybir.dt.int16
BF = ml_dtypes.bfloat16

NCORES = 8
D = 64
L = 3
EPS = 1e-5
BLK = 128
MAXT_CALL = 32          # max tiles (128 idx each) per dma_gather call
SCRATCH = 49152         # dynamic_dma_scratch_size (SWDGE ring = 3072 descs)
NWIN = 3                # x_full windows (= staggered AllGathers per layer)


def _cdiv(a, b):
    return (a + b - 1) // b


def _cfg(n):
    assert n % (2 * NCORES) == 0
    P = n // NCORES              # nodes per core
    assert P % 2 == 0
    NB = _cdiv(P, BLK)           # blocks per core
    NBP = _cdiv(NB, 4)           # regions (4 blocks) per core
    # window block splits: first ~22% (early AllGather), rest even
    nb0 = min(NB - (NWIN - 1), max(1, int(round(0.22 * NB))))
    rest = NB - nb0
    wblk = [nb0]
    for i in range(1, NWIN):
        k = rest // (NWIN - 1) + (1 if i <= rest % (NWIN - 1) else 0)
        wblk.append(k)
    assert sum(wblk) == NB and all(k >= 1 for k in wblk)
    wb0 = np.concatenate([[0], np.cumsum(wblk)])     # block range starts
    # rows per core per window (last window absorbs the P remainder)
    whp = [wblk[w] * BLK for w in range(NWIN)]
    whp[-1] = P - sum(whp[:-1])
    assert all(h % 2 == 0 and h > 0 for h in whp)
    hpsum = np.concatenate([[0], np.cumsum(whp)])    # per-core row starts
    wsz = [NCORES * h // 2 for h in whp]             # pair rows per window
    assert all(wv <= 32767 for wv in wsz), "gather idx must fit int16"
    wbase = np.concatenate([[0], np.cumsum(wsz)])    # pair base per window
    rbase = NCORES * hpsum                           # x_full row base
    return dict(P=P, NB=NB, NBP=NBP, WBLK=wblk, WB0=wb0, WHP=whp,
                HPSUM=hpsum, WSZ=wsz, WBASE=wbase, RBASE=rbase)


def _assign_nodes(edge_dst, n, cfg):
    """Balanced node -> (core, position) assignment: snake-deal nodes in
    descending in-degree order across the NCORES*NBP block-pair bins."""
    P, NBP = cfg["P"], cfg["NBP"]
    indeg = np.bincount(edge_dst, minlength=n).astype(np.int64)
    order = np.argsort(-indeg, kind="stable")

    nbins = NCORES * NBP
    cap_last = P - (NBP - 1) * 512
    caps = np.full(nbins, 512, np.int64)
    caps[NBP - 1::NBP] = cap_last
    fill = np.zeros(nbins, np.int64)

    core_of = np.empty(n, np.int32)
    pos_of = np.empty(n, np.int32)
    taken = 0
    direction = 1
    while taken < n:
        active = np.nonzero(fill < caps)[0]
        if direction < 0:
            active = active[::-1]
        direction = -direction
        k = min(len(active), n - taken)
        active = active[:k]
        nodes = order[taken:taken + k]
        core = (active // NBP).astype(np.int32)
        bpair = active % NBP
        core_of[nodes] = core
        pos_of[nodes] = (bpair * 512 + fill[active]).astype(np.int32)
        fill[active] += 1
        taken += k
    return core_of, pos_of


def _row_of_pos(core, pos, cfg):
    """(core, position) -> x_full row (windows concatenated by rank)."""
    hpsum, whp, rbase = cfg["HPSUM"], cfg["WHP"], cfg["RBASE"]
    pos = np.asarray(pos, np.int64)
    c = np.asarray(core, np.int64)
    w = np.searchsorted(hpsum[1:], pos, side="right")
    return rbase[w] + c * np.asarray(whp)[w] + (pos - hpsum[w])


def _preprocess(edge_src, edge_dst, n_nodes):
    cfg = _cfg(n_nodes)
    P, NB, NBP = cfg["P"], cfg["NB"], cfg["NBP"]
    wbase = cfg["WBASE"]

    core_of, pos_of = _assign_nodes(edge_dst, n_nodes, cfg)
    row_of = _row_of_pos(core_of, pos_of, cfg)

    deg = np.bincount(edge_dst, minlength=n_nodes).astype(np.float32)
    inv_deg = np.where(deg > 0, 1.0 / np.maximum(deg, 1.0), 0.0).astype(
        np.float32)

    srow = row_of[edge_src]                      # x_full row of source
    spair = srow // 2
    swin = np.searchsorted(wbase[1:], spair, side="right")
    spar = (srow % 2).astype(np.int64)           # parity (slab half)
    sidx = spair - wbase[swin]                   # int16 gather idx

    dcore = core_of[edge_dst]
    dpos = pos_of[edge_dst].astype(np.int64)
    dbp = dpos // 512
    dslot = dpos % 512

    # per-core sorted edge runs keyed by (bpair, window, parity)
    counts = np.zeros((NCORES, NBP, NWIN, 2), np.int64)
    cores = []
    for c in range(NCORES):
        m = dcore == c
        bp, w, pr = dbp[m], swin[m], spar[m]
        si, sl = sidx[m], dslot[m]
        iv = inv_deg[edge_dst[m]]
        order = np.lexsort((sl, pr, w, bp))
        bp, w, pr, si, sl, iv = (bp[order], w[order], pr[order], si[order],
                                 sl[order], iv[order])
        np.add.at(counts[c], (bp, w, pr), 1)
        cores.append((bp, w, pr, si, sl, iv))

    ntiles = _cdiv(counts.max(axis=0), BLK)      # [NBP, NWIN, 2]

    # tile & call emission order: bpair -> window -> parity
    tiles = []       # (bpair, window, parity, call)
    calls = []       # (bpair, window, tile_off, ntiles)
    for r in range(NBP):
        for w in range(NWIN):
            nt_w = int(ntiles[r, w, :].sum())
            if nt_w == 0:
                continue
            ncalls = _cdiv(nt_w, MAXT_CALL)
            sizes = [nt_w // ncalls + (1 if i < nt_w % ncalls else 0)
                     for i in range(ncalls)]
            flat = [p for p in range(2) for _ in range(int(ntiles[r, w, p]))]
            done = 0
            for sz in sizes:
                calls.append(dict(bpair=r, window=w, tile_off=len(tiles),
                                  ntiles=sz))
                for p in flat[done:done + sz]:
                    tiles.append(dict(bpair=r, window=w, parity=p,
                                      call=len(calls) - 1))
                done += sz
    T = len(tiles)
    NIDX = T * BLK

    # per-core lane tables
    gidx_all, slots_all, invd_all = [], [], []
    for c in range(NCORES):
        bp, w, pr, si, sl, iv = cores[c]
        run_start = {}
        cum = 0
        for r in range(NBP):
            for ww in range(NWIN):
                for p in range(2):
                    run_start[(r, ww, p)] = cum
                    cum += counts[c, r, ww, p]
        consumed = {}
        idx_flat = np.zeros(NIDX, np.int16)
        slot_flat = np.full(NIDX, -1.0, np.float32)
        invd_flat = np.zeros(NIDX, np.float32)
        for ti, t in enumerate(tiles):
            key = (t["bpair"], t["window"], t["parity"])
            got = consumed.get(key, 0)
            nrem = counts[c, key[0], key[1], key[2]] - got
            nfill = int(min(BLK, max(nrem, 0)))
            if nfill > 0:
                e0 = run_start[key] + got
                sel = slice(e0, e0 + nfill)
                base = ti * BLK
                idx_flat[base:base + nfill] = si[sel].astype(np.int16)
                slot_flat[base:base + nfill] = sl[sel].astype(np.float32)
                invd_flat[base:base + nfill] = iv[sel]
                consumed[key] = got + nfill
        s = idx_flat.reshape(NIDX // 16, 16)
        gi = np.zeros((128, NIDX // 16), np.int16)
        for grp in range(8):
            gi[grp * 16:(grp + 1) * 16, :] = s.T
        gidx_all.append(gi)
        slots_all.append(slot_flat.reshape(T, BLK).T.copy())
        invd_all.append(invd_flat.reshape(T, BLK).T.copy())

    # per-tile dst-slot band (uniform across cores) -> narrow selectors
    allsl = np.stack(slots_all)                      # [NCORES, 128, T]
    pad = allsl < 0
    mn = np.where(pad, 512, allsl).min(axis=(0, 1)).astype(np.int64)
    mx = np.where(pad, -1, allsl).max(axis=(0, 1)).astype(np.int64)
    width = np.maximum(mx - mn + 1, 1)
    selw = np.minimum(256, ((width + 31) // 32) * 32)
    seloff = np.minimum(np.maximum(mn, 0), 512 - selw)
    assert (width <= 256).all(), "selector band exceeds bf16-exact range"

    for c in range(NCORES):
        sl = slots_all[c]
        p = sl < 0
        sl -= seloff[None, :].astype(np.float32)
        sl[p] = -1.0

    return dict(cfg=cfg, tiles=tiles, calls=calls, T=T, NIDX=NIDX,
                gidx=gidx_all, slots=slots_all, invd=invd_all,
                selw=selw, seloff=seloff,
                core_of=core_of, pos_of=pos_of)


def _build_nc(meta):
    cfg = meta["cfg"]
    P, NB, NBP = cfg["P"], cfg["NB"], cfg["NBP"]
    WSZ, WBASE, WB0, WBLK = cfg["WSZ"], cfg["WBASE"], cfg["WB0"], cfg["WBLK"]
    HPSUM = cfg["HPSUM"]
    T = meta["T"]
    tiles, calls = meta["tiles"], meta["calls"]
    XPC = NB * BLK                  # padded per-core node columns in xT
    NPAIR = int(WBASE[-1])

    nc = bacc.Bacc("TRN2", target_bir_lowering=False, debug=False,
                   num_devices=NCORES, dynamic_dma_scratch_size=SCRATCH)

    # ---- I/O ----
    xloc_d = nc.dram_tensor("xloc", [P, D], BF16, kind="ExternalInput")
    gidx_d = nc.dram_tensor("gidx", [16, meta["NIDX"] // 16], I16,
                            kind="ExternalInput")
    slots_d = nc.dram_tensor("slots", [128, T], BF16, kind="ExternalInput")
    invd_d = nc.dram_tensor("invd", [128, T], BF16, kind="ExternalInput")
    iota_d = nc.dram_tensor("iota", [128, 256], BF16, kind="ExternalInput")
    ident_d = nc.dram_tensor("ident", [128, 128], BF16, kind="ExternalInput")
    wl_d = nc.dram_tensor("wl", [D, L * D], BF16, kind="ExternalInput")
    wr_d = nc.dram_tensor("wr", [D, L * D], BF16, kind="ExternalInput")
    wres_d = nc.dram_tensor("wres", [D, D], BF16, kind="ExternalInput")
    wfc_d = nc.dram_tensor("wfc", [D, D], BF16, kind="ExternalInput")
    blrep_d = nc.dram_tensor("blrep", [128, L * D], F32, kind="ExternalInput")
    garep_d = nc.dram_tensor("garep", [128, L * D], F32, kind="ExternalInput")
    berep_d = nc.dram_tensor("berep", [128, L * D], F32, kind="ExternalInput")
    bresrep_d = nc.dram_tensor("bresrep", [128, D], F32, kind="ExternalInput")
    bfcrep_d = nc.dram_tensor("bfcrep", [128, D], F32, kind="ExternalInput")
    out_d = nc.dram_tensor("out", [P, D], BF16, kind="ExternalOutput")

    with tile.TileContext(nc) as tc, ExitStack() as ctx:
        dram = ctx.enter_context(tc.tile_pool(name="dram", bufs=1,
                                              space="DRAM"))
        singles = ctx.enter_context(tc.tile_pool(name="singles", bufs=1))
        slabp = ctx.enter_context(tc.tile_pool(name="slabp", bufs=3))
        selp = ctx.enter_context(tc.tile_pool(name="selp", bufs=6))
        aggsb = ctx.enter_context(tc.tile_pool(name="aggsb", bufs=3))
        blkp = ctx.enter_context(tc.tile_pool(name="blkp", bufs=4))
        lnp = ctx.enter_context(tc.tile_pool(name="lnp", bufs=4))
        asb0p = ctx.enter_context(tc.tile_pool(name="asb0p", bufs=2))
        aggps = ctx.enter_context(tc.tile_pool(name="aggps", bufs=2,
                                               space="PSUM"))
        hps = ctx.enter_context(tc.tile_pool(name="hps", bufs=2,
                                             space="PSUM"))
        tps = ctx.enter_context(tc.tile_pool(name="tps", bufs=2,
                                             space="PSUM"))
        rfps = ctx.enter_context(tc.tile_pool(name="rfps", bufs=2,
                                              space="PSUM"))

        # internal DRAM: contrib (AllGather input) + xf ping-pong (output)
        contrib = [dram.tile([P, D], BF16, name=f"contrib{i}",
                             tag=f"contrib{i}") for i in range(2)]
        xf = [[dram.tile([WSZ[w], 2 * D], BF16, name=f"xf{i}w{w}",
                         tag=f"xf{i}w{w}", addr_space="Shared")
               for w in range(NWIN)] for i in range(2)]
        xfi = [dram.tile([WSZ[w], 2 * D], BF16, name=f"xfiw{w}",
                         tag=f"xfiw{w}", addr_space="Shared")
               for w in range(NWIN)]
        cinit = dram.tile([P, D], BF16, name="cinit", tag="cinit")

        # ---- resident SBUF ----
        iota_sb = singles.tile([128, 256], BF16)
        nc.sync.dma_start(iota_sb[:], iota_d[:, :])
        ident_sb = singles.tile([128, 128], BF16)
        nc.sync.dma_start(ident_sb[:], ident_d[:, :])
        gidx_sb = singles.tile([128, meta["NIDX"] // 16], I16)
        for g in range(8):
            nc.sync.dma_start(gidx_sb[g * 16:(g + 1) * 16, :], gidx_d[:, :])
        slots_bf = singles.tile([128, T], BF16)
        nc.sync.dma_start(slots_bf[:], slots_d[:, :])
        slots_sb = singles.tile([128, T], F32)
        nc.scalar.activation(slots_sb[:], slots_bf[:],
                             mybir.ActivationFunctionType.Copy)
        invd_bf = singles.tile([128, T], BF16)
        nc.sync.dma_start(invd_bf[:], invd_d[:, :])
        invd_sb = singles.tile([128, T], F32)
        nc.scalar.activation(invd_sb[:], invd_bf[:],
                             mybir.ActivationFunctionType.Copy)
        wl_sb = singles.tile([D, L * D], BF16)
        nc.sync.dma_start(wl_sb[:], wl_d[:, :])
        wr_sb = singles.tile([D, L * D], BF16)
        nc.sync.dma_start(wr_sb[:], wr_d[:, :])
        wres_sb = singles.tile([D, D], BF16)
        nc.sync.dma_start(wres_sb[:], wres_d[:, :])
        wfc_sb = singles.tile([D, D], BF16)
        nc.sync.dma_start(wfc_sb[:], wfc_d[:, :])
        blrep_sb = singles.tile([128, L * D], F32)
        nc.sync.dma_start(blrep_sb[:], blrep_d[:, :])
        garep_sb = singles.tile([128, L * D], F32)
        nc.sync.dma_start(garep_sb[:], garep_d[:, :])
        berep_sb = singles.tile([128, L * D], F32)
        nc.sync.dma_start(berep_sb[:], berep_d[:, :])
        bresrep_sb = singles.tile([128, D], F32)
        nc.sync.dma_start(bresrep_sb[:], bresrep_d[:, :])
        bfcrep_sb = singles.tile([128, D], F32)
        nc.sync.dma_start(bfcrep_sb[:], bfcrep_d[:, :])
        eps_sb = singles.tile([128, 1], F32)
        nc.vector.memset(eps_sb[:], EPS)
        zmm1 = singles.tile([1, D], BF16)
        nc.vector.memset(zmm1[:], 0.0)
        zmm2 = singles.tile([1, 512], BF16)
        nc.vector.memset(zmm2[:], 0.0)

        xT_sb = singles.tile([D, XPC], BF16)      # feature-major x (current)
        xnat_sb = singles.tile([128, NB, D], BF16)  # node-major x (bf16)
        # last block whose contrib write completes each window
        win_last_block = [int(WB0[w + 1]) - 1 for w in range(NWIN)]

        # distribute layer-0 x: bulk-copy xloc to internal DRAM, then
        # AllGather each window immediately (collectives can't read IO)
        for w in range(NWIN):
            h0, h1 = int(HPSUM[w]), int(HPSUM[w + 1])
            nc.sync.dma_start(cinit[h0:h1, :], xloc_d[h0:h1, :])
            nc.gpsimd.collective_compute(
                "AllGather",
                mybir.AluOpType.bypass,
                replica_groups=[list(range(NCORES))],
                ins=[cinit[h0:h1, :].opt()],
                outs=[xfi[w][:, :].opt()],
            )

        # load x_local into xnat, build xT via PE transposes
        for b in range(NB):
            nrow = min(BLK, P - b * BLK)
            if nrow < BLK:
                nc.vector.memset(xnat_sb[:, b, :], 0.0)
            nc.sync.dma_start(xnat_sb[:nrow, b, :],
                              xloc_d[b * BLK:b * BLK + nrow, :])
            tp = tps.tile([D, 128], BF16, tag="tp")
            nc.tensor.transpose(tp[:], xnat_sb[:, b, :], ident_sb[:])
            nc.scalar.activation(xT_sb[:, b * BLK:(b + 1) * BLK], tp[:],
                                 mybir.ActivationFunctionType.Copy)

        # per-(bpair, window) tile lists
        tiles_of = {}
        for ti, t in enumerate(tiles):
            tiles_of.setdefault((t["bpair"], t["window"]), []).append(ti)

        selw, seloff = meta["selw"], meta["seloff"]

        def emit_window(layer, r, w, at):
            """Gathers + selector matmuls of (bpair r, window w) into PSUM."""
            wts = tiles_of.get((r, w))
            if not wts:
                return False
            sl = slabp.tile([128, len(wts), 2 * D], BF16, tag="slab")
            for cl in calls:
                if cl["bpair"] != r or cl["window"] != w:
                    continue
                nt = cl["ntiles"]
                off = cl["tile_off"] - wts[0]
                src_ap = (xfi[w] if layer == 0
                          else xf[(layer + 1) % 2][w])[:, :]
                nc.gpsimd.dma_gather(
                    out_ap=sl[:, off:off + nt, :],
                    in_ap=src_ap,
                    idxs_ap=gidx_sb[:, cl["tile_off"] * 8:
                                    (cl["tile_off"] + nt) * 8],
                    num_idxs=nt * BLK,
                    num_idxs_reg=nt * BLK,
                    elem_size=2 * D,
                    single_packet=False,
                )
            # claim/zero the full region so narrow matmuls accumulate
            nc.tensor.matmul(at[:, :], zmm1[:], zmm2[:],
                             start=True, stop=False, skip_group_check=True)
            for k, ti in enumerate(wts):
                w_t = int(selw[ti])
                o_t = int(seloff[ti])
                sel = selp.tile([128, w_t], BF16, tag="sel")
                nc.vector.tensor_scalar(
                    out=sel[:], in0=iota_sb[:, 0:w_t],
                    scalar1=slots_sb[:, ti:ti + 1],
                    scalar2=invd_sb[:, ti:ti + 1],
                    op0=mybir.AluOpType.is_equal,
                    op1=mybir.AluOpType.mult)
                pr = tiles[ti]["parity"]
                nc.tensor.matmul(
                    at[:, o_t:o_t + w_t],
                    sl[:, ti - wts[0], pr * D:(pr + 1) * D],
                    sel[:],
                    start=False, stop=(k == len(wts) - 1),
                    skip_group_check=True)
            return True

        for layer in range(L):
            asb0 = asb0p.tile([D, NBP, 512], BF16, tag="asb0")
            # staging phases: windows 0..NWIN-2
            for ph in range(NWIN - 1):
                for r in range(NBP):
                    at = aggps.tile([D, 512], F32, tag="agg")
                    have = emit_window(layer, r, ph, at)
                    if ph == 0:
                        if have:
                            nc.scalar.activation(
                                asb0[:, r, :], at[:, :],
                                mybir.ActivationFunctionType.Copy)
                        else:
                            nc.vector.memset(asb0[:, r, :], 0.0)
                    elif have:
                        nc.vector.tensor_add(asb0[:, r, :], at[:, :],
                                             asb0[:, r, :])
            # final phase: last window + combine + block pipeline
            for r in range(NBP):
                at = aggps.tile([D, 512], F32, tag="agg")
                asb = aggsb.tile([D, 512], BF16, tag="aggsb")
                if emit_window(layer, r, NWIN - 1, at):
                    nc.vector.tensor_add(asb[:], at[:, :], asb0[:, r, :])
                else:
                    nc.scalar.activation(asb[:], asb0[:, r, :],
                                         mybir.ActivationFunctionType.Copy)
                # per-block pipeline
                for half in range(4):
                    b = 4 * r + half
                    if b >= NB:
                        continue
                    nrow = min(BLK, P - b * BLK)
                    ht = hps.tile([128, D], F32, tag="h")
                    nc.tensor.matmul(
                        ht[:, :], asb[:, half * BLK:half * BLK + BLK],
                        wl_sb[:, layer * D:(layer + 1) * D],
                        start=True, stop=False, skip_group_check=True)
                    nc.tensor.matmul(
                        ht[:, :], xT_sb[:, b * BLK:(b + 1) * BLK],
                        wr_sb[:, layer * D:(layer + 1) * D],
                        start=False, stop=True, skip_group_check=True)
                    if layer == 0:
                        rf = rfps.tile([128, D], F32, tag="rf")
                        nc.tensor.matmul(
                            rf[:, :], xT_sb[:, b * BLK:(b + 1) * BLK],
                            wres_sb[:, :], start=True, stop=True)
                        res_sb = blkp.tile([128, D], BF16, tag="res")
                        nc.vector.tensor_add(res_sb[:], rf[:, :],
                                             bresrep_sb[:])
                    # h = ht + b_l ; LayerNorm ; relu ; + residual
                    hsb = lnp.tile([128, D], F32, tag="hsb")
                    nc.vector.tensor_add(
                        hsb[:], ht[:, :],
                        blrep_sb[:, layer * D:(layer + 1) * D])
                    st = lnp.tile([128, 6], F32, tag="st")
                    nc.vector.bn_stats(out=st[:], in_=hsb[:])
                    mv = lnp.tile([128, 2], F32, tag="mv")
                    nc.vector.bn_aggr(out=mv[:], in_=st[:])
                    rs_t = lnp.tile([128, 1], F32, tag="rs")
                    nc.scalar.activation(rs_t[:], mv[:, 1:2],
                                         mybir.ActivationFunctionType.Sqrt,
                                         bias=eps_sb[:])
                    nc.vector.reciprocal(rs_t[:], rs_t[:])
                    nsb = lnp.tile([128, D], F32, tag="nsb")
                    nc.vector.tensor_scalar(
                        out=nsb[:], in0=hsb[:],
                        scalar1=mv[:, 0:1], scalar2=rs_t[:],
                        op0=mybir.AluOpType.subtract,
                        op1=mybir.AluOpType.mult)
                    nc.gpsimd.tensor_mul(
                        nsb[:], nsb[:],
                        garep_sb[:, layer * D:(layer + 1) * D])
                    nc.gpsimd.tensor_add(
                        nsb[:], nsb[:],
                        berep_sb[:, layer * D:(layer + 1) * D])
                    rlu = blkp.tile([128, D], BF16, tag="rlu")
                    nc.scalar.activation(rlu[:], nsb[:],
                                         mybir.ActivationFunctionType.Relu)
                    if layer == 0:
                        nc.vector.tensor_add(xnat_sb[:, b, :], rlu[:],
                                             res_sb[:])
                    else:
                        nc.vector.tensor_add(xnat_sb[:, b, :], rlu[:],
                                             xnat_sb[:, b, :])
                    # new xT column block
                    tp = tps.tile([D, 128], BF16, tag="tp")
                    nc.tensor.transpose(tp[:], xnat_sb[:, b, :], ident_sb[:])
                    nc.scalar.activation(
                        xT_sb[:, b * BLK:(b + 1) * BLK], tp[:],
                        mybir.ActivationFunctionType.Copy)
                    if layer < L - 1:
                        cb = contrib[layer % 2]
                        nc.sync.dma_start(
                            cb[b * BLK:b * BLK + nrow, :],
                            xnat_sb[:nrow, b, :])
                        for w in range(NWIN):
                            if win_last_block[w] != b:
                                continue
                            h0, h1 = int(HPSUM[w]), int(HPSUM[w + 1])
                            nc.gpsimd.collective_compute(
                                "AllGather",
                                mybir.AluOpType.bypass,
                                replica_groups=[list(range(NCORES))],
                                ins=[cb[h0:h1, :].opt()],
                                outs=[xf[layer % 2][w][:, :].opt()],
                            )
                    else:
                        fc = rfps.tile([128, D], F32, tag="rf")
                        nc.tensor.matmul(
                            fc[:, :], xT_sb[:, b * BLK:(b + 1) * BLK],
                            wfc_sb[:, :], start=True, stop=True)
                        osb = blkp.tile([128, D], BF16, tag="osb")
                        nc.vector.tensor_add(osb[:], fc[:, :], bfcrep_sb[:])
                        nc.sync.dma_start(
                            out_d[b * BLK:b * BLK + nrow, :],
                            osb[:nrow, :])
    nc.compile()
    return nc


_CACHE = {}


def _get_compiled(edge_src, edge_dst, n_nodes):
    key = hashlib.sha1(edge_src.tobytes() + edge_dst.tobytes()).hexdigest()
    if key not in _CACHE:
        meta = _preprocess(edge_src, edge_dst, n_nodes)
        nc = _build_nc(meta)
        _CACHE[key] = (meta, nc)
    return _CACHE[key]


def _host_inputs(meta, x, w_l, b_l, w_r, gamma, beta, w_res, b_res, w_fc,
                 b_fc):
    cfg = meta["cfg"]
    P = cfg["P"]
    n = x.shape[0]
    core_of, pos_of = meta["core_of"], meta["pos_of"]

    xloc = np.zeros((NCORES, P, D), np.float32)
    xloc[core_of, pos_of] = x
    xloc = xloc.astype(BF)

    rep = lambda v: np.broadcast_to(
        v.reshape(1, -1), (128, v.size)).astype(np.float32).copy()
    wl = np.concatenate([w_l[i] for i in range(L)], axis=1).astype(BF)
    wr = np.concatenate([w_r[i] for i in range(L)], axis=1).astype(BF)
    iota = np.broadcast_to(np.arange(256, dtype=np.float32),
                           (128, 256)).astype(BF).copy()
    ident = np.eye(128, dtype=np.float32).astype(BF)

    in_maps = []
    for c in range(NCORES):
        in_maps.append(dict(
            xloc=xloc[c],
            gidx=meta["gidx"][c][:16],
            slots=meta["slots"][c].astype(BF),
            invd=meta["invd"][c].astype(BF),
            iota=iota, ident=ident,
            wl=wl, wr=wr,
            wres=w_res.astype(BF), wfc=w_fc.astype(BF),
            blrep=rep(b_l.reshape(-1)), garep=rep(gamma.reshape(-1)),
            berep=rep(beta.reshape(-1)),
            bresrep=rep(b_res), bfcrep=rep(b_fc),
        ))
    return in_maps


def kernel(x, edge_src, edge_dst, w_l, b_l, w_r, gamma, beta, w_res, b_res,
           w_fc, b_fc, _want_trace=False):
    x = np.asarray(x, np.float32)
    edge_src = np.asarray(edge_src, np.int32)
    edge_dst = np.asarray(edge_dst, np.int32)
    n = x.shape[0]
    meta, nc = _get_compiled(edge_src, edge_dst, n)
    in_maps = _host_inputs(meta, x, np.asarray(w_l), np.asarray(b_l),
                           np.asarray(w_r), np.asarray(gamma),
                           np.asarray(beta), np.asarray(w_res),
                           np.asarray(b_res), np.asarray(w_fc),
                           np.asarray(b_fc))
    try:
        res = run_bass_kernel_spmd(nc, in_maps, core_ids=list(range(NCORES)),
                                   trace=_want_trace)
    except ModuleNotFoundError:
        res = run_bass_kernel_spmd(nc, in_maps, core_ids=list(range(NCORES)),
                                   trace=False)
    core_of, pos_of = meta["core_of"], meta["pos_of"]
    out = np.empty((n, D), np.float32)
    percore = np.stack([np.asarray(res.results[c]["out"], np.float32)
                        for c in range(NCORES)])
    out[:] = percore[core_of, pos_of]
    if _want_trace:
        kernel._last_results = res
    return out
